# revision 33
# baseline (speedup 1.0000x reference)
"""BertCrossAttention (relative_key_query) Trainium2 kernel.

Full inputs -> full output. Sharding: 8 cores, core c handles batch b=c//2 and
heads [8*(c%2), 8*(c%2)+8). All sharding/slicing/transposition happens on the
host; each core runs an identical Bass program on its own slices.

Math (per core, per head h), with a global score scale S=16 folded in so the
small relative-position tables survive fp8 storage:
  q = (xq @ Wq^T + bq) * S/8        [Lq=1024, 64] fp16   (1/sqrt(64)=1/8)
  k = x @ Wk^T + bk                  [Lk=2048, 64] fp16
  v = x @ Wv^T + bv                  [Lk=2048, 65] fp16 (ones column appended)
  S[l,r'] = q.k + q.E[t] + k.(S/8 E)[t],  t = l + r'   (key axis host-reversed)
  probs = exp(S/S + mask[r'])  (mask via ACT exp bias; softmax denom from the
                                ones column of v)
  out[l, dh] = (probs @ v)[l, dh] / rowsum[l]

All matmul inputs fp16 (1 PE cycle/row vs 4 for fp32); KE stored fp8e4.

Key layout trick: the key axis is REVERSED on the host (r' = 2047 - r), making
t = l + r'. Then with QE[l,t] (fp16) and KE[r',t] (fp8, k.E pre-scaled) stored
in DRAM as dense windowed blocks, both rel-score reads become plain 2D strided
DMAs (row stride = width+1 "skew trick" on flat DRAM):
  rel1[l, r'] = QE[l, l+r']   (cast-read as [l-part, r'-free] fp32 tiles, then
                               PE-transpose-accumulated onto QK scores in PSUM)
  rel2^T[r', l] = KE[r', l+r'] (read directly as [r'-part, l-free] fp8 tiles,
                               added into PSUM via an identity matmul)
Scores are built transposed (S^T [r'-part, l-free]) so PV needs no transpose
of the probabilities. Table builds run one head ahead of the score loop, with
each block's read-back issued immediately after its write so the DRAM
round-trip overlaps the previous head's score matmuls. The final ctx
transposes go through the DMA XBAR (SBUF->SBUF), not PSUM.

q/k/v stay SBUF-resident between projections and attention: projection output
tile m ([128ch, l]) holds heads 2m (partitions 0-63) and 2m+1 (64-127);
per-head matmuls slice at partition base 0/64 (PE quadrant addressing), with
the E tables duplicated into both halves.
"""

import os
import sys
from contextlib import ExitStack

import numpy as np

sys.path.insert(0, "/opt/trn_rl_repo")

import concourse.bass as bass
import concourse.mybir as mybir
import concourse.tile as tile
from concourse import bacc
from concourse.masks import make_identity

F32 = mybir.dt.float32
F16 = mybir.dt.float16
F8 = mybir.dt.float8e4

B, H, DH, D = 4, 16, 64, 1024
LQ, LK = 1024, 2048
HPC = 8              # heads per core
CH = HPC * DH        # 512 output channels per core
TW = 3072            # E-table columns used (t in [0, 3071))
QW = 2176            # QE block storage width (cols 0..2174 used, 2175 pad)
KW = 1151            # KE block storage width (cols 0..1150 used)
NKT = D // 128       # 8 contraction tiles for projections
SCALE = 16.0         # global score scale (pre-folded into q and the KE table)


def build_nc():
    nc = bacc.Bacc("TRN2", target_bir_lowering=False, debug=False, num_devices=8)

    xqT = nc.dram_tensor("xqT", [D, LQ], F16, kind="ExternalInput")
    xT = nc.dram_tensor("xT", [D, LK], F16, kind="ExternalInput")
    wqT = nc.dram_tensor("wqT", [D, CH], F16, kind="ExternalInput")
    wkT = nc.dram_tensor("wkT", [D, CH], F16, kind="ExternalInput")
    wvT = nc.dram_tensor("wvT", [D, CH], F16, kind="ExternalInput")
    bqv = nc.dram_tensor("bqv", [CH], F32, kind="ExternalInput")
    bkv = nc.dram_tensor("bkv", [CH], F32, kind="ExternalInput")
    bvv = nc.dram_tensor("bvv", [CH], F32, kind="ExternalInput")
    eT = nc.dram_tensor("eT", [128, TW], F16, kind="ExternalInput")
    esT = nc.dram_tensor("esT", [128, TW], F16, kind="ExternalInput")
    maskc = nc.dram_tensor("maskc", [128, 16], F32, kind="ExternalInput")
    out = nc.dram_tensor("out", [LQ, CH], F32, kind="ExternalOutput")

    with tile.TileContext(nc) as tc, ExitStack() as ctx:
        const = ctx.enter_context(tc.tile_pool(name="const", bufs=1))
        ident_f = const.tile([128, 128], F32)
        make_identity(nc, ident_f)
        ident_8 = const.tile([128, 128], F8)
        make_identity(nc, ident_8)
        # E tables, duplicated into both partition halves (host-prepared)
        et_sb = const.tile([128, TW], F16, tag="et")
        es_sb = const.tile([128, TW], F16, tag="est")
        nc.sync.dma_start(et_sb, eT[:, :])
        nc.sync.dma_start(es_sb, esT[:, :])
        mask_sb = const.tile([128, 16], F32, tag="mask")
        nc.sync.dma_start(mask_sb, maskc[:, :])
        bq_sb = const.tile([128, 4], F32, tag="bq")
        bk_sb = const.tile([128, 4], F32, tag="bk")
        nc.sync.dma_start(bq_sb, bqv.rearrange("(t p) -> p t", p=128))
        nc.sync.dma_start(bk_sb, bkv.rearrange("(t p) -> p t", p=128))
        bv_sb = const.tile([128, HPC, DH], F32, tag="bv")
        nc.sync.dma_start(
            bv_sb, bass.AP(tensor=bvv, offset=0, ap=[[0, 128], [DH, HPC], [1, DH]])
        )

        # persistent per-core activation tensors (SBUF-resident q/k/v)
        persist = ctx.enter_context(tc.tile_pool(name="persist", bufs=1))
        q_sb = [
            persist.tile([128, LQ], F16, tag=f"q{m}", name=f"q_sb{m}")
            for m in range(4)
        ]
        k_sb = [
            persist.tile([128, LK], F16, tag=f"k{m}", name=f"k_sb{m}")
            for m in range(4)
        ]
        v_sb = persist.tile([128, 16, HPC, DH + 1], F16, tag="v")  # [r',j,h,dh|1]
        ctx_all = persist.tile([128, 8, CH], F32, tag="ctxo")  # [l%128, lblk, ch]
        nc.vector.memset(v_sb[:, :, :, DH], 1.0)

        # ---------------- Phase 1: projections (fp16) ----------------
        proj = ExitStack()
        ppool = proj.enter_context(tc.tile_pool(name="pp", bufs=2, space="PSUM"))
        ppv = proj.enter_context(tc.tile_pool(name="ppv", bufs=2, space="PSUM"))
        pact = proj.enter_context(tc.tile_pool(name="pact", bufs=1))

        xq_sb, wq_sb = [], []
        for t in range(NKT):
            xt_full = pact.tile([128, LK], F16, tag=f"x{t}", name="xt_full")
            xt = xt_full[:, 0:LQ]
            wt = pact.tile([128, CH], F16, tag=f"w{t}", name="wt")
            nc.sync.dma_start(xt, xqT[128 * t:128 * (t + 1), :])
            nc.sync.dma_start(wt, wqT[128 * t:128 * (t + 1), :])
            xq_sb.append(xt)
            wq_sb.append(wt)

        # Q projection (matmul outputs are capped at 512 fp32 elements, so
        # each 1024-wide PSUM tile fills in halves; one bias-copy drains it)
        for m in range(4):
            ps = ppool.tile([128, LQ], F32, tag="pp", name="ps")
            for n in range(2):
                for t in range(NKT):
                    nc.tensor.matmul(
                        ps[:, 512 * n:512 * (n + 1)],
                        wq_sb[t][:, 128 * m:128 * (m + 1)],
                        xq_sb[t][:, 512 * n:512 * (n + 1)],
                        start=(t == 0), stop=(t == NKT - 1),
                    )
            nc.scalar.activation(
                q_sb[m], ps,
                mybir.ActivationFunctionType.Identity,
                bias=bq_sb[:, m:m + 1],
            )

        x_sb, wk_sb = [], []
        for t in range(NKT):
            xt = pact.tile([128, LK], F16, tag=f"x{t}", name="xt")
            nc.sync.dma_start(xt, xT[128 * t:128 * (t + 1), :])
            x_sb.append(xt)
        for t in range(NKT):
            wt = pact.tile([128, CH], F16, tag=f"w{t}", name="wt")
            nc.sync.dma_start(wt, wkT[128 * t:128 * (t + 1), :])
            wk_sb.append(wt)

        # K projection
        for m in range(4):
            for n in range(2):
                ps = ppool.tile([128, LQ], F32, tag="pp", name="ps")
                for u in range(2):
                    for t in range(NKT):
                        nc.tensor.matmul(
                            ps[:, 512 * u:512 * (u + 1)],
                            wk_sb[t][:, 128 * m:128 * (m + 1)],
                            x_sb[t][:, LQ * n + 512 * u:LQ * n + 512 * (u + 1)],
                            start=(t == 0), stop=(t == NKT - 1),
                        )
                nc.scalar.activation(
                    k_sb[m][:, LQ * n:LQ * (n + 1)], ps,
                    mybir.ActivationFunctionType.Identity,
                    bias=bk_sb[:, m:m + 1],
                )

        # V projection, natural layout: out[r', ch]; lhsT = xT r'-slice
        wv_sb = []
        for t in range(NKT):
            wt = pact.tile([128, CH], F16, tag=f"w{t}", name="wt")
            nc.sync.dma_start(wt, wvT[128 * t:128 * (t + 1), :])
            wv_sb.append(wt)
        for j in range(16):         # r' tiles of 128
            ps3 = ppv.tile([128, HPC, DH], F32, tag="ppv", name="ps3")
            for t in range(NKT):
                nc.tensor.matmul(
                    ps3,
                    x_sb[t][:, 128 * j:128 * (j + 1)],
                    wv_sb[t],
                    start=(t == 0), stop=(t == NKT - 1),
                )
            nc.vector.tensor_add(v_sb[:, j, :, 0:DH], ps3, bv_sb)

        # ---------------- phase boundary ----------------
        proj.close()
        with tc.tile_critical():
            nc.all_engine_barrier()

        # ---------------- Phase 2: attention per head ----------------
        qe_dram = ctx.enter_context(tc.tile_pool(name="qed", bufs=16, space="DRAM"))
        ke_dram = ctx.enter_context(tc.tile_pool(name="ked", bufs=32, space="DRAM"))
        relp = ctx.enter_context(tc.tile_pool(name="relp", bufs=4))
        rel1p = ctx.enter_context(tc.tile_pool(name="rel1p", bufs=10))
        r2p = ctx.enter_context(tc.tile_pool(name="r2p", bufs=40))
        ptp = ctx.enter_context(tc.tile_pool(name="ptp", bufs=3))
        cnp = ctx.enter_context(tc.tile_pool(name="cnp", bufs=2))
        qeps = ctx.enter_context(tc.tile_pool(name="qeps", bufs=3, space="PSUM"))
        sps = ctx.enter_context(tc.tile_pool(name="sps", bufs=3, space="PSUM"))
        cps = ctx.enter_context(tc.tile_pool(name="cps", bufs=1, space="PSUM"))

        def build_head(h):
            """Emit QE/KE table builds + rel1/rel2 read-backs for head h.
            Called one head ahead of the score loop; each block's read-back
            is issued right after its write so the round-trip streams."""
            m, hb = h // 2, h % 2
            b0 = 64 * hb            # partition base of this head's q/k rows
            qh = q_sb[m]
            kh = k_sb[m]

            # QE blocks: QE[l, t] fp16, l-block i holds t-window [l0, l0+2175);
            # rel1 cast-read (fp16 -> fp32, skew stride QW+1) follows each write
            rel1_sb = []
            for i in range(8):
                l0 = 128 * i
                qe_sb = relp.tile([128, QW], F16, tag="qe_sb", name="qe_sb")
                for c, w in ((0, 512), (512, 512), (1024, 512), (1536, 512), (2048, 127)):
                    ps = qeps.tile([128, 512], F32, tag="qeps", name="ps")
                    nc.tensor.matmul(
                        ps[:, 0:w],
                        qh[b0:b0 + 64, l0:l0 + 128],
                        et_sb[b0:b0 + 64, l0 + c:l0 + c + w],
                        start=True, stop=True,
                    )
                    if c < 1536:
                        nc.vector.tensor_copy(qe_sb[:, c:c + w], ps[:, 0:w])
                    else:
                        nc.scalar.activation(
                            qe_sb[:, c:c + w], ps[:, 0:w],
                            mybir.ActivationFunctionType.Identity,
                        )
                qe_t = qe_dram.tile([128, QW], F16, tag="qe_d", name="qe_t")
                nc.sync.dma_start(qe_t[:, 0:QW - 1], qe_sb[:, 0:QW - 1])
                t1 = rel1p.tile([128, LK], F32, tag="rel1", name="t1")
                src = bass.AP(
                    tensor=qe_t.tensor,
                    offset=qe_t.offset,
                    ap=[[QW + 1, 128], [1, LK]],
                )
                nc.gpsimd.dma_start(out=t1, in_=src)
                rel1_sb.append(t1)

            # KE blocks: KE[r', t] fp8, window [r0, r0+1151); both rel2^T
            # halves (skew stride KW+1) follow each write
            r2_tiles = []
            for j in range(16):
                r0 = 128 * j
                ke_sb = relp.tile([128, KW], F8, tag="ke_sb", name="ke_sb")
                for c, w in ((0, 512), (512, 512), (1024, 127)):
                    ps = qeps.tile([128, 512], F32, tag="qeps", name="ps")
                    nc.tensor.matmul(
                        ps[:, 0:w],
                        kh[b0:b0 + 64, r0:r0 + 128],
                        es_sb[b0:b0 + 64, r0 + c:r0 + c + w],
                        start=True, stop=True,
                    )
                    if c < 1024:
                        nc.vector.tensor_copy(ke_sb[:, c:c + w], ps[:, 0:w])
                    else:
                        nc.scalar.activation(
                            ke_sb[:, c:c + w], ps[:, 0:w],
                            mybir.ActivationFunctionType.Identity,
                        )
                ke_t = ke_dram.tile([128, KW], F8, tag="ke_d", name="ke_t")
                nc.sync.dma_start(ke_t, ke_sb)
                for lh in range(2):
                    r2 = r2p.tile([128, 512], F8, tag="r2", name="r2")
                    src = bass.AP(
                        tensor=ke_t.tensor,
                        offset=ke_t.offset + 512 * lh,
                        ap=[[KW + 1, 128], [1, 512]],
                    )
                    nc.gpsimd.dma_start(out=r2, in_=src)
                    r2_tiles.append(r2)
            return rel1_sb, r2_tiles

        def score_head(h, built):
            m, hb = h // 2, h % 2
            b0 = 64 * hb
            qh = q_sb[m]
            kh = k_sb[m]
            rel1_sb, r2_tiles = built

            ctx_ps = cps.tile([DH + 1, LQ], F32, tag="ctxps", name="ctx_ps")
            for lh in range(2):
                for j in range(16):
                    s_ps = sps.tile([128, 512], F32, tag="sps", name="s_ps")
                    # QK^T: [r' 128, l 512]
                    nc.tensor.matmul(
                        s_ps,
                        kh[b0:b0 + 64, 128 * j:128 * (j + 1)],
                        qh[b0:b0 + 64, 512 * lh:512 * (lh + 1)],
                        start=True, stop=False,
                    )
                    # rel1: PE-transpose-accumulate 4 blocks of this l-half
                    for ii in range(4):
                        i = 4 * lh + ii
                        nc.tensor.matmul(
                            s_ps[:, 128 * ii:128 * (ii + 1)],
                            rel1_sb[i][:, 128 * j:128 * (j + 1)],
                            ident_f,
                            is_transpose=True,
                            start=False, stop=False,
                        )
                    # rel2: added into PSUM via I @ r2 on the PE
                    nc.tensor.matmul(
                        s_ps, ident_8, r2_tiles[2 * j + lh],
                        start=False, stop=True,
                    )
                    # probs = exp(scores/S + mask[r']), mask per-partition
                    pt = ptp.tile([128, 512], F16, tag="pt", name="pt")
                    nc.scalar.activation(
                        pt, s_ps, mybir.ActivationFunctionType.Exp,
                        bias=mask_sb[:, j:j + 1], scale=1.0 / SCALE,
                    )
                    nc.tensor.matmul(
                        ctx_ps[:, 512 * lh:512 * (lh + 1)],
                        v_sb[:, j, h, :],
                        pt,
                        start=(j == 0), stop=(j == 15),
                    )

            # ctx+rowsum -> SBUF fp16 (80 partitions so the XBAR transpose's
            # 16-row tiling works); transpose [65, 128]-chunks to [128, 80]
            # via SBUF->SBUF XBAR; 1/rowsum applied on the ACT copy out.
            cn_sb = cnp.tile([80, LQ], F16, tag="ctxn", name="cn_sb")
            nc.vector.tensor_copy(cn_sb[0:DH + 1, :], ctx_ps)
            for i in range(8):
                ct = cnp.tile([128, 80], F16, tag="ct", name="ct", bufs=2)
                nc.scalar.dma_start_transpose(ct, cn_sb[:, 128 * i:128 * (i + 1)])
                rs_inv = cnp.tile([128, 1], F32, tag="rsinv", name="rs_inv")
                nc.vector.reciprocal(rs_inv, ct[:, DH:DH + 1])
                nc.scalar.activation(
                    ctx_all[:, i, DH * h:DH * (h + 1)], ct[:, 0:DH],
                    mybir.ActivationFunctionType.Copy,
                    scale=rs_inv,
                )

        built = build_head(0)
        for h in range(HPC):
            nxt = build_head(h + 1) if h + 1 < HPC else None
            score_head(h, built)
            built = nxt

        nc.sync.dma_start(out.rearrange("(i p) c -> p i c", p=128), ctx_all)

    nc.compile()
    return nc


def make_in_maps(inputs):
    hs = np.asarray(inputs["hidden_states"], np.float32)
    qhs = np.asarray(inputs["query_hidden_states"], np.float32)
    am = np.asarray(inputs["attention_mask"], np.float32)
    Wq = np.asarray(inputs["Wq"], np.float32)
    bq = np.asarray(inputs["bq"], np.float32)
    Wk = np.asarray(inputs["Wk"], np.float32)
    bk = np.asarray(inputs["bk"], np.float32)
    Wv = np.asarray(inputs["Wv"], np.float32)
    bv = np.asarray(inputs["bv"], np.float32)
    de = np.asarray(inputs["dist_emb"], np.float32)

    # E^T tables, duplicated into both partition halves (base-0 and base-64
    # head slots); esT pre-scaled by S/8 for the KE side.
    eT = np.zeros((128, TW), np.float32)
    eT[0:DH, :3071] = de[:3071].T
    eT[DH:128, :3071] = de[:3071].T
    esT = eT * (SCALE / 8.0)

    qs = SCALE / 8.0
    in_maps = []
    for core in range(8):
        b = core // 2
        hg = core % 2
        sl = slice(CH * hg, CH * (hg + 1))
        m16 = {
            "xqT": np.ascontiguousarray(qhs[b].T),
            "xT": np.ascontiguousarray(hs[b].T[:, ::-1]),
            "wqT": np.ascontiguousarray(Wq[sl].T) * qs,
            "wkT": np.ascontiguousarray(Wk[sl].T),
            "wvT": np.ascontiguousarray(Wv[sl].T),
            "eT": eT,
            "esT": esT,
        }
        m32 = {
            "bqv": np.ascontiguousarray(bq[sl]) * qs,
            "bkv": np.ascontiguousarray(bk[sl]),
            "bvv": np.ascontiguousarray(bv[sl]),
            "maskc": np.ascontiguousarray(am[b, 0, 0, ::-1].reshape(16, 128).T),
        }
        m = {k: np.ascontiguousarray(v.astype(np.float16)) for k, v in m16.items()}
        m.update({k: np.ascontiguousarray(v.astype(np.float32)) for k, v in m32.items()})
        in_maps.append(m)
    return in_maps


_CACHED = {}


def kernel(**inputs):
    from concourse.bass_utils import run_bass_kernel_spmd

    if "nc" not in _CACHED:
        _CACHED["nc"] = build_nc()
    nc = _CACHED["nc"]
    in_maps = make_in_maps(inputs)
    res = run_bass_kernel_spmd(nc, in_maps, list(range(8)))
    _CACHED["last_result"] = res
    out = np.zeros((B, LQ, D), np.float32)
    for core in range(8):
        b = core // 2
        hg = core % 2
        out[b, :, CH * hg:CH * (hg + 1)] = res.results[core]["out"]
    return out


# revision 34
# speedup vs baseline: 1.1269x; 1.1269x over previous
"""BertCrossAttention (relative_key_query) Trainium2 kernel.

Full inputs -> full output. Sharding: 8 cores, core c handles batch b=c//2 and
heads [8*(c%2), 8*(c%2)+8). All sharding/slicing/transposition happens on the
host; each core runs an identical Bass program on its own slices.

Math (per core, per head h), with a global score scale S=16 folded in so the
small relative-position tables survive fp8 storage:
  q = (xq @ Wq^T + bq) * S/8        [Lq=1024, 64] fp16   (1/sqrt(64)=1/8)
  k = x @ Wk^T + bk                  [Lk=2048, 64] fp16
  v = x @ Wv^T + bv                  [Lk=2048, 65] fp16 (ones column appended)
  S[l,r'] = q.k + q.E[t] + k.(S/8 E)[t],  t = l + r'   (key axis host-reversed)
  probs = exp(S/S + mask[r'])  (mask via ACT exp bias; softmax denom from the
                                ones column of v)
  out[l, dh] = (probs @ v)[l, dh] / rowsum[l]

All matmul inputs fp16 (1 PE cycle/row vs 4 for fp32); KE stored fp8e4.

Key layout trick: the key axis is REVERSED on the host (r' = 2047 - r), making
t = l + r'. Then with QE[l,t] (fp16) and KE[r',t] (fp8, k.E pre-scaled) stored
in DRAM as dense windowed blocks, both rel-score reads become plain 2D strided
DMAs (row stride = width+1 "skew trick" on flat DRAM):
  rel1[l, r'] = QE[l, l+r']   (cast-read as [l-part, r'-free] fp32 tiles, then
                               PE-transpose-accumulated onto QK scores in PSUM)
  rel2^T[r', l] = KE[r', l+r'] (read directly as [r'-part, l-free] fp8 tiles,
                               added into PSUM via an identity matmul)
Scores are built transposed (S^T [r'-part, l-free]) so PV needs no transpose
of the probabilities. Table builds run one head ahead of the score loop, with
each block's read-back issued immediately after its write so the DRAM
round-trip overlaps the previous head's score matmuls. The final ctx
transposes go through the DMA XBAR (SBUF->SBUF), not PSUM.

q/k/v stay SBUF-resident between projections and attention: projection output
tile m ([128ch, l]) holds heads 2m (partitions 0-63) and 2m+1 (64-127);
per-head matmuls slice at partition base 0/64 (PE quadrant addressing), with
the E tables duplicated into both halves.
"""

import os
import sys
from contextlib import ExitStack

import numpy as np

sys.path.insert(0, "/opt/trn_rl_repo")

import concourse.bass as bass
import concourse.mybir as mybir
import concourse.tile as tile
from concourse import bacc
from concourse.masks import make_identity

F32 = mybir.dt.float32
F16 = mybir.dt.float16
F8 = mybir.dt.float8e4

B, H, DH, D = 4, 16, 64, 1024
LQ, LK = 1024, 2048
HPC = 8              # heads per core
CH = HPC * DH        # 512 output channels per core
TW = 3072            # E-table columns used (t in [0, 3071))
QW = 2176            # QE block storage width (cols 0..2174 used, 2175 pad)
KW = 1151            # KE block storage width (cols 0..1150 used)
NKT = D // 128       # 8 contraction tiles for projections
SCALE = 16.0         # global score scale (pre-folded into q and the KE table)


def build_nc():
    nc = bacc.Bacc("TRN2", target_bir_lowering=False, debug=False, num_devices=8)

    xqT = nc.dram_tensor("xqT", [D, LQ], F16, kind="ExternalInput")
    xT = nc.dram_tensor("xT", [D, LK], F16, kind="ExternalInput")
    wqT = nc.dram_tensor("wqT", [D, CH], F16, kind="ExternalInput")
    wkT = nc.dram_tensor("wkT", [D, CH], F16, kind="ExternalInput")
    wvT = nc.dram_tensor("wvT", [D, CH], F16, kind="ExternalInput")
    bqv = nc.dram_tensor("bqv", [CH], F32, kind="ExternalInput")
    bkv = nc.dram_tensor("bkv", [CH], F32, kind="ExternalInput")
    bvv = nc.dram_tensor("bvv", [CH], F32, kind="ExternalInput")
    eT = nc.dram_tensor("eT", [128, TW], F16, kind="ExternalInput")
    esT = nc.dram_tensor("esT", [128, TW], F16, kind="ExternalInput")
    maskc = nc.dram_tensor("maskc", [128, 16], F32, kind="ExternalInput")
    out = nc.dram_tensor("out", [LQ, CH], F32, kind="ExternalOutput")

    with tile.TileContext(nc) as tc, ExitStack() as ctx:
        const = ctx.enter_context(tc.tile_pool(name="const", bufs=1))
        ident_f = const.tile([128, 128], F32)
        make_identity(nc, ident_f)
        ident_8 = const.tile([128, 128], F8)
        make_identity(nc, ident_8)
        # E tables, duplicated into both partition halves (host-prepared)
        et_sb = const.tile([128, TW], F16, tag="et")
        es_sb = const.tile([128, TW], F16, tag="est")
        nc.sync.dma_start(et_sb, eT[:, :])
        nc.sync.dma_start(es_sb, esT[:, :])
        mask_sb = const.tile([128, 16], F32, tag="mask")
        nc.sync.dma_start(mask_sb, maskc[:, :])
        bq_sb = const.tile([128, 4], F32, tag="bq")
        bk_sb = const.tile([128, 4], F32, tag="bk")
        nc.sync.dma_start(bq_sb, bqv.rearrange("(t p) -> p t", p=128))
        nc.sync.dma_start(bk_sb, bkv.rearrange("(t p) -> p t", p=128))
        bv_sb = const.tile([128, HPC, DH], F32, tag="bv")
        nc.sync.dma_start(
            bv_sb, bass.AP(tensor=bvv, offset=0, ap=[[0, 128], [DH, HPC], [1, DH]])
        )

        # persistent per-core activation tensors (SBUF-resident q/k/v)
        persist = ctx.enter_context(tc.tile_pool(name="persist", bufs=1))
        q_sb = [
            persist.tile([128, LQ], F16, tag=f"q{m}", name=f"q_sb{m}")
            for m in range(4)
        ]
        k_sb = [
            persist.tile([128, LK], F16, tag=f"k{m}", name=f"k_sb{m}")
            for m in range(4)
        ]
        v_sb = persist.tile([128, 16, HPC, DH + 1], F16, tag="v")  # [r',j,h,dh|1]
        ctx_all = persist.tile([128, 8, CH], F32, tag="ctxo")  # [l%128, lblk, ch]
        nc.vector.memset(v_sb[:, :, :, DH], 1.0)

        # ---------------- Phase 1: projections (fp16) ----------------
        proj = ExitStack()
        ppool = proj.enter_context(tc.tile_pool(name="pp", bufs=2, space="PSUM"))
        ppv = proj.enter_context(tc.tile_pool(name="ppv", bufs=2, space="PSUM"))
        pact = proj.enter_context(tc.tile_pool(name="pact", bufs=1))

        xq_sb, wq_sb = [], []
        for t in range(NKT):
            xt_full = pact.tile([128, LK], F16, tag=f"x{t}", name="xt_full")
            xt = xt_full[:, 0:LQ]
            wt = pact.tile([128, CH], F16, tag=f"w{t}", name="wt")
            nc.sync.dma_start(xt, xqT[128 * t:128 * (t + 1), :])
            nc.sync.dma_start(wt, wqT[128 * t:128 * (t + 1), :])
            xq_sb.append(xt)
            wq_sb.append(wt)

        # Q projection (matmul outputs are capped at 512 fp32 elements, so
        # each 1024-wide PSUM tile fills in halves; one bias-copy drains it)
        for m in range(4):
            ps = ppool.tile([128, LQ], F32, tag="pp", name="ps")
            for n in range(2):
                for t in range(NKT):
                    nc.tensor.matmul(
                        ps[:, 512 * n:512 * (n + 1)],
                        wq_sb[t][:, 128 * m:128 * (m + 1)],
                        xq_sb[t][:, 512 * n:512 * (n + 1)],
                        start=(t == 0), stop=(t == NKT - 1),
                    )
            nc.scalar.activation(
                q_sb[m], ps,
                mybir.ActivationFunctionType.Identity,
                bias=bq_sb[:, m:m + 1],
            )

        x_sb, wk_sb = [], []
        for t in range(NKT):
            xt = pact.tile([128, LK], F16, tag=f"x{t}", name="xt")
            nc.sync.dma_start(xt, xT[128 * t:128 * (t + 1), :])
            x_sb.append(xt)
        for t in range(NKT):
            wt = pact.tile([128, CH], F16, tag=f"w{t}", name="wt")
            nc.sync.dma_start(wt, wkT[128 * t:128 * (t + 1), :])
            wk_sb.append(wt)

        # K projection
        for m in range(4):
            for n in range(2):
                ps = ppool.tile([128, LQ], F32, tag="pp", name="ps")
                for u in range(2):
                    for t in range(NKT):
                        nc.tensor.matmul(
                            ps[:, 512 * u:512 * (u + 1)],
                            wk_sb[t][:, 128 * m:128 * (m + 1)],
                            x_sb[t][:, LQ * n + 512 * u:LQ * n + 512 * (u + 1)],
                            start=(t == 0), stop=(t == NKT - 1),
                        )
                nc.scalar.activation(
                    k_sb[m][:, LQ * n:LQ * (n + 1)], ps,
                    mybir.ActivationFunctionType.Identity,
                    bias=bk_sb[:, m:m + 1],
                )

        # V projection, natural layout: out[r', ch]; lhsT = xT r'-slice
        wv_sb = []
        for t in range(NKT):
            wt = pact.tile([128, CH], F16, tag=f"w{t}", name="wt")
            nc.sync.dma_start(wt, wvT[128 * t:128 * (t + 1), :])
            wv_sb.append(wt)
        for j in range(16):         # r' tiles of 128
            ps3 = ppv.tile([128, HPC, DH], F32, tag="ppv", name="ps3")
            for t in range(NKT):
                nc.tensor.matmul(
                    ps3,
                    x_sb[t][:, 128 * j:128 * (j + 1)],
                    wv_sb[t],
                    start=(t == 0), stop=(t == NKT - 1),
                )
            nc.vector.tensor_add(v_sb[:, j, :, 0:DH], ps3, bv_sb)

        # ---------------- phase boundary ----------------
        proj.close()
        with tc.tile_critical():
            nc.all_engine_barrier()

        # ---------------- Phase 2: attention per head ----------------
        qe_dram = ctx.enter_context(tc.tile_pool(name="qed", bufs=16, space="DRAM"))
        ke_dram = ctx.enter_context(tc.tile_pool(name="ked", bufs=32, space="DRAM"))
        relp = ctx.enter_context(tc.tile_pool(name="relp", bufs=3))
        rel1p = ctx.enter_context(tc.tile_pool(name="rel1p", bufs=10))
        r2p = ctx.enter_context(tc.tile_pool(name="r2p", bufs=40))
        ptp = ctx.enter_context(tc.tile_pool(name="ptp", bufs=3))
        cnp = ctx.enter_context(tc.tile_pool(name="cnp", bufs=2))
        qeps = ctx.enter_context(tc.tile_pool(name="qeps", bufs=3, space="PSUM"))
        sps = ctx.enter_context(tc.tile_pool(name="sps", bufs=3, space="PSUM"))
        cps = ctx.enter_context(tc.tile_pool(name="cps", bufs=1, space="PSUM"))

        def build_head(h):
            """Emit QE/KE table builds + rel1/rel2 read-backs for head h.
            Called one head ahead of the score loop; each block's read-back
            is issued right after its write so the round-trip streams."""
            m, hb = h // 2, h % 2
            b0 = 64 * hb            # partition base of this head's q/k rows
            qh = q_sb[m]
            kh = k_sb[m]

            # QE blocks: QE[l, t] fp16, l-block i holds t-window [l0, l0+2175);
            # rel1 cast-read (fp16 -> fp32, skew stride QW+1) follows each write
            rel1_sb = []
            for i in range(8):
                l0 = 128 * i
                qe_sb = relp.tile([128, QW], F16, tag="qe_sb", name="qe_sb")
                for c, w in ((0, 512), (512, 512), (1024, 512), (1536, 512), (2048, 127)):
                    ps = qeps.tile([128, 512], F32, tag="qeps", name="ps")
                    nc.tensor.matmul(
                        ps[:, 0:w],
                        qh[b0:b0 + 64, l0:l0 + 128],
                        et_sb[b0:b0 + 64, l0 + c:l0 + c + w],
                        start=True, stop=True,
                    )
                    if c < 1536:
                        nc.vector.tensor_copy(qe_sb[:, c:c + w], ps[:, 0:w])
                    else:
                        nc.scalar.activation(
                            qe_sb[:, c:c + w], ps[:, 0:w],
                            mybir.ActivationFunctionType.Identity,
                        )
                qe_t = qe_dram.tile([128, QW], F16, tag="qe_d", name="qe_t")
                nc.sync.dma_start(qe_t[:, 0:QW - 1], qe_sb[:, 0:QW - 1])
                t1 = rel1p.tile([128, LK], F32, tag="rel1", name="t1")
                src = bass.AP(
                    tensor=qe_t.tensor,
                    offset=qe_t.offset,
                    ap=[[QW + 1, 128], [1, LK]],
                )
                nc.gpsimd.dma_start(out=t1, in_=src)
                rel1_sb.append(t1)

            # KE blocks: KE[r', t] fp8, window [r0, r0+1151); both rel2^T
            # halves (skew stride KW+1) follow each write
            r2_tiles = []
            for j in range(16):
                r0 = 128 * j
                ke_sb = relp.tile([128, KW], F8, tag="ke_sb", name="ke_sb")
                for c, w in ((0, 512), (512, 512), (1024, 127)):
                    ps = qeps.tile([128, 512], F32, tag="qeps", name="ps")
                    nc.tensor.matmul(
                        ps[:, 0:w],
                        kh[b0:b0 + 64, r0:r0 + 128],
                        es_sb[b0:b0 + 64, r0 + c:r0 + c + w],
                        start=True, stop=True,
                    )
                    if c < 1024:
                        nc.vector.tensor_copy(ke_sb[:, c:c + w], ps[:, 0:w])
                    else:
                        nc.scalar.activation(
                            ke_sb[:, c:c + w], ps[:, 0:w],
                            mybir.ActivationFunctionType.Identity,
                        )
                ke_t = ke_dram.tile([128, KW], F8, tag="ke_d", name="ke_t")
                nc.sync.dma_start(ke_t, ke_sb)
                for lh in range(2):
                    r2 = r2p.tile([128, 512], F8, tag="r2", name="r2")
                    src = bass.AP(
                        tensor=ke_t.tensor,
                        offset=ke_t.offset + 512 * lh,
                        ap=[[KW + 1, 128], [1, 512]],
                    )
                    nc.sync.dma_start(out=r2, in_=src)
                    r2_tiles.append(r2)
            return rel1_sb, r2_tiles

        def score_head(h, built):
            m, hb = h // 2, h % 2
            b0 = 64 * hb
            qh = q_sb[m]
            kh = k_sb[m]
            rel1_sb, r2_tiles = built

            ctx_ps = cps.tile([DH + 1, LQ], F32, tag="ctxps", name="ctx_ps")
            for lh in range(2):
                for j in range(16):
                    s_ps = sps.tile([128, 512], F32, tag="sps", name="s_ps")
                    # QK^T: [r' 128, l 512]
                    nc.tensor.matmul(
                        s_ps,
                        kh[b0:b0 + 64, 128 * j:128 * (j + 1)],
                        qh[b0:b0 + 64, 512 * lh:512 * (lh + 1)],
                        start=True, stop=False,
                    )
                    # rel1: PE-transpose-accumulate 4 blocks of this l-half
                    for ii in range(4):
                        i = 4 * lh + ii
                        nc.tensor.matmul(
                            s_ps[:, 128 * ii:128 * (ii + 1)],
                            rel1_sb[i][:, 128 * j:128 * (j + 1)],
                            ident_f,
                            is_transpose=True,
                            start=False, stop=False,
                        )
                    # rel2: added into PSUM via I @ r2 on the PE
                    nc.tensor.matmul(
                        s_ps, ident_8, r2_tiles[2 * j + lh],
                        start=False, stop=True,
                    )
                    # probs = exp(scores/S + mask[r']), mask per-partition
                    pt = ptp.tile([128, 512], F16, tag="pt", name="pt")
                    nc.scalar.activation(
                        pt, s_ps, mybir.ActivationFunctionType.Exp,
                        bias=mask_sb[:, j:j + 1], scale=1.0 / SCALE,
                    )
                    nc.tensor.matmul(
                        ctx_ps[:, 512 * lh:512 * (lh + 1)],
                        v_sb[:, j, h, :],
                        pt,
                        start=(j == 0), stop=(j == 15),
                    )

            # ctx+rowsum -> SBUF fp16 (80 partitions so the XBAR transpose's
            # 16-row tiling works); transpose [65, 128]-chunks to [128, 80]
            # via SBUF->SBUF XBAR; 1/rowsum applied on the ACT copy out.
            cn_sb = cnp.tile([80, LQ], F16, tag="ctxn", name="cn_sb")
            nc.vector.tensor_copy(cn_sb[0:DH + 1, :], ctx_ps)
            for i in range(8):
                ct = cnp.tile([128, 80], F16, tag="ct", name="ct", bufs=2)
                nc.sync.dma_start_transpose(ct, cn_sb[:, 128 * i:128 * (i + 1)])
                rs_inv = cnp.tile([128, 1], F32, tag="rsinv", name="rs_inv")
                nc.vector.reciprocal(rs_inv, ct[:, DH:DH + 1])
                nc.scalar.activation(
                    ctx_all[:, i, DH * h:DH * (h + 1)], ct[:, 0:DH],
                    mybir.ActivationFunctionType.Copy,
                    scale=rs_inv,
                )

        built = build_head(0)
        for h in range(HPC):
            nxt = build_head(h + 1) if h + 1 < HPC else None
            score_head(h, built)
            built = nxt

        nc.sync.dma_start(out.rearrange("(i p) c -> p i c", p=128), ctx_all)

    nc.compile()
    return nc


def make_in_maps(inputs):
    hs = np.asarray(inputs["hidden_states"], np.float32)
    qhs = np.asarray(inputs["query_hidden_states"], np.float32)
    am = np.asarray(inputs["attention_mask"], np.float32)
    Wq = np.asarray(inputs["Wq"], np.float32)
    bq = np.asarray(inputs["bq"], np.float32)
    Wk = np.asarray(inputs["Wk"], np.float32)
    bk = np.asarray(inputs["bk"], np.float32)
    Wv = np.asarray(inputs["Wv"], np.float32)
    bv = np.asarray(inputs["bv"], np.float32)
    de = np.asarray(inputs["dist_emb"], np.float32)

    # E^T tables, duplicated into both partition halves (base-0 and base-64
    # head slots); esT pre-scaled by S/8 for the KE side.
    eT = np.zeros((128, TW), np.float32)
    eT[0:DH, :3071] = de[:3071].T
    eT[DH:128, :3071] = de[:3071].T
    esT = eT * (SCALE / 8.0)

    qs = SCALE / 8.0
    in_maps = []
    for core in range(8):
        b = core // 2
        hg = core % 2
        sl = slice(CH * hg, CH * (hg + 1))
        m16 = {
            "xqT": np.ascontiguousarray(qhs[b].T),
            "xT": np.ascontiguousarray(hs[b].T[:, ::-1]),
            "wqT": np.ascontiguousarray(Wq[sl].T) * qs,
            "wkT": np.ascontiguousarray(Wk[sl].T),
            "wvT": np.ascontiguousarray(Wv[sl].T),
            "eT": eT,
            "esT": esT,
        }
        m32 = {
            "bqv": np.ascontiguousarray(bq[sl]) * qs,
            "bkv": np.ascontiguousarray(bk[sl]),
            "bvv": np.ascontiguousarray(bv[sl]),
            "maskc": np.ascontiguousarray(am[b, 0, 0, ::-1].reshape(16, 128).T),
        }
        m = {k: np.ascontiguousarray(v.astype(np.float16)) for k, v in m16.items()}
        m.update({k: np.ascontiguousarray(v.astype(np.float32)) for k, v in m32.items()})
        in_maps.append(m)
    return in_maps


_CACHED = {}


def kernel(**inputs):
    from concourse.bass_utils import run_bass_kernel_spmd

    if "nc" not in _CACHED:
        _CACHED["nc"] = build_nc()
    nc = _CACHED["nc"]
    in_maps = make_in_maps(inputs)
    res = run_bass_kernel_spmd(nc, in_maps, list(range(8)))
    _CACHED["last_result"] = res
    out = np.zeros((B, LQ, D), np.float32)
    for core in range(8):
        b = core // 2
        hg = core % 2
        out[b, :, CH * hg:CH * (hg + 1)] = res.results[core]["out"]
    return out


# revision 41
# speedup vs baseline: 1.1379x; 1.0098x over previous
"""BertCrossAttention (relative_key_query) Trainium2 kernel.

Full inputs -> full output. Sharding: 8 cores, core c handles batch b=c//2 and
heads [8*(c%2), 8*(c%2)+8). All sharding/slicing/transposition happens on the
host; each core runs an identical Bass program on its own slices.

Math (per core, per head h), with a global score scale S=16 folded in so the
small relative-position tables survive fp8 storage:
  q = (xq @ Wq^T + bq) * S/8        [Lq=1024, 64] fp16   (1/sqrt(64)=1/8)
  k = x @ Wk^T + bk                  [Lk=2048, 64] fp16
  v = x @ Wv^T + bv                  [Lk=2048, 65] fp16 (ones column appended)
  S[l,r'] = q.k + q.E[t] + k.(S/8 E)[t],  t = l + r'   (key axis host-reversed)
  probs = exp(S/S + mask[r'])  (mask via ACT exp bias; softmax denom from the
                                ones column of v)
  out[l, dh] = (probs @ v)[l, dh] / rowsum[l]

All matmul inputs fp16 (1 PE cycle/row vs 4 for fp32); KE stored fp8e4.

Key layout trick: the key axis is REVERSED on the host (r' = 2047 - r), making
t = l + r'. Then with QE[l,t] (fp16) and KE[r',t] (fp8, k.E pre-scaled) stored
in DRAM as dense windowed blocks, both rel-score reads become plain 2D strided
DMAs (row stride = width+1 "skew trick" on flat DRAM):
  rel1[l, r'] = QE[l, l+r']   (cast-read as [l-part, r'-free] fp32 tiles, then
                               PE-transpose-accumulated onto QK scores in PSUM)
  rel2^T[r', l] = KE[r', l+r'] (read directly as [r'-part, l-free] fp8 tiles,
                               added into PSUM via an identity matmul)
Scores are built transposed (S^T [r'-part, l-free]) so PV needs no transpose
of the probabilities. Table builds run one head ahead of the score loop, with
each block's read-back issued immediately after its write so the DRAM
round-trip overlaps the previous head's score matmuls. The final ctx
transposes go through the DMA XBAR (SBUF->SBUF), not PSUM.

q/k/v stay SBUF-resident between projections and attention: projection output
tile m ([128ch, l]) holds heads 2m (partitions 0-63) and 2m+1 (64-127);
per-head matmuls slice at partition base 0/64 (PE quadrant addressing), with
the E tables duplicated into both halves.
"""

import os
import sys
from contextlib import ExitStack

import numpy as np

sys.path.insert(0, "/opt/trn_rl_repo")

import concourse.bass as bass
import concourse.mybir as mybir
import concourse.tile as tile
from concourse import bacc
from concourse.masks import make_identity

F32 = mybir.dt.float32
F16 = mybir.dt.float16
F8 = mybir.dt.float8e4

B, H, DH, D = 4, 16, 64, 1024
LQ, LK = 1024, 2048
HPC = 8              # heads per core
CH = HPC * DH        # 512 output channels per core
TW = 3072            # E-table columns used (t in [0, 3071))
QW = 2176            # QE block storage width (cols 0..2174 used, 2175 pad)
KW = 1151            # KE block storage width (cols 0..1150 used)
NKT = D // 128       # 8 contraction tiles for projections
SCALE = 16.0         # global score scale (pre-folded into q and the KE table)


def build_nc():
    nc = bacc.Bacc("TRN2", target_bir_lowering=False, debug=False, num_devices=8)

    xqT = nc.dram_tensor("xqT", [D, LQ], F16, kind="ExternalInput")
    xT = nc.dram_tensor("xT", [D, LK], F16, kind="ExternalInput")
    wqT = nc.dram_tensor("wqT", [D, CH], F16, kind="ExternalInput")
    wkT = nc.dram_tensor("wkT", [D, CH], F16, kind="ExternalInput")
    wvT = nc.dram_tensor("wvT", [D, CH], F16, kind="ExternalInput")
    bqv = nc.dram_tensor("bqv", [CH], F32, kind="ExternalInput")
    bkv = nc.dram_tensor("bkv", [CH], F32, kind="ExternalInput")
    bvv = nc.dram_tensor("bvv", [CH], F32, kind="ExternalInput")
    eT = nc.dram_tensor("eT", [128, TW], F16, kind="ExternalInput")
    esT = nc.dram_tensor("esT", [128, TW], F16, kind="ExternalInput")
    maskc = nc.dram_tensor("maskc", [128, 16], F32, kind="ExternalInput")
    out = nc.dram_tensor("out", [LQ, CH], F32, kind="ExternalOutput")

    with tile.TileContext(nc) as tc, ExitStack() as ctx:
        const = ctx.enter_context(tc.tile_pool(name="const", bufs=1))
        ident_f = const.tile([128, 128], F32)
        make_identity(nc, ident_f)
        ident_8 = const.tile([128, 128], F8)
        make_identity(nc, ident_8)
        # E tables, duplicated into both partition halves (host-prepared)
        et_sb = const.tile([128, TW], F16, tag="et")
        es_sb = const.tile([128, TW], F16, tag="est")
        nc.sync.dma_start(et_sb, eT[:, :])
        nc.sync.dma_start(es_sb, esT[:, :])
        mask_sb = const.tile([128, 16], F32, tag="mask")
        nc.sync.dma_start(mask_sb, maskc[:, :])
        bq_sb = const.tile([128, 4], F32, tag="bq")
        bk_sb = const.tile([128, 4], F32, tag="bk")
        nc.sync.dma_start(bq_sb, bqv.rearrange("(t p) -> p t", p=128))
        nc.sync.dma_start(bk_sb, bkv.rearrange("(t p) -> p t", p=128))
        bv_sb = const.tile([128, HPC, DH], F32, tag="bv")
        nc.sync.dma_start(
            bv_sb, bass.AP(tensor=bvv, offset=0, ap=[[0, 128], [DH, HPC], [1, DH]])
        )

        # persistent per-core activation tensors (SBUF-resident q/k/v)
        persist = ctx.enter_context(tc.tile_pool(name="persist", bufs=1))
        q_sb = [
            persist.tile([128, LQ], F16, tag=f"q{m}", name=f"q_sb{m}")
            for m in range(4)
        ]
        k_sb = [
            persist.tile([128, LK], F16, tag=f"k{m}", name=f"k_sb{m}")
            for m in range(4)
        ]
        v_sb = persist.tile([128, 16, HPC, DH + 1], F16, tag="v")  # [r',j,h,dh|1]
        ctx_all = persist.tile([128, 8, CH], F32, tag="ctxo")  # [l%128, lblk, ch]
        nc.vector.memset(v_sb[:, :, :, DH], 1.0)

        # ---------------- Phase 1: projections (fp16) ----------------
        proj = ExitStack()
        ppool = proj.enter_context(tc.tile_pool(name="pp", bufs=2, space="PSUM"))
        ppv = proj.enter_context(tc.tile_pool(name="ppv", bufs=2, space="PSUM"))
        pact = proj.enter_context(tc.tile_pool(name="pact", bufs=1))

        xq_sb, wq_sb = [], []
        for t in range(NKT):
            xt_full = pact.tile([128, LK], F16, tag=f"x{t}", name="xt_full")
            xt = xt_full[:, 0:LQ]
            wt = pact.tile([128, CH], F16, tag=f"w{t}", name="wt")
            nc.sync.dma_start(xt, xqT[128 * t:128 * (t + 1), :])
            nc.sync.dma_start(wt, wqT[128 * t:128 * (t + 1), :])
            xq_sb.append(xt)
            wq_sb.append(wt)

        # Q projection (matmul outputs are capped at 512 fp32 elements, so
        # each 1024-wide PSUM tile fills in halves; one bias-copy drains it)
        for m in range(4):
            ps = ppool.tile([128, LQ], F32, tag="pp", name="ps")
            for n in range(2):
                for t in range(NKT):
                    nc.tensor.matmul(
                        ps[:, 512 * n:512 * (n + 1)],
                        wq_sb[t][:, 128 * m:128 * (m + 1)],
                        xq_sb[t][:, 512 * n:512 * (n + 1)],
                        start=(t == 0), stop=(t == NKT - 1),
                    )
            nc.scalar.activation(
                q_sb[m], ps,
                mybir.ActivationFunctionType.Identity,
                bias=bq_sb[:, m:m + 1],
            )

        x_sb, wk_sb = [], []
        for t in range(NKT):
            xt = pact.tile([128, LK], F16, tag=f"x{t}", name="xt")
            nc.sync.dma_start(xt, xT[128 * t:128 * (t + 1), :])
            x_sb.append(xt)
        for t in range(NKT):
            wt = pact.tile([128, CH], F16, tag=f"w{t}", name="wt")
            nc.sync.dma_start(wt, wkT[128 * t:128 * (t + 1), :])
            wk_sb.append(wt)

        # K projection
        for m in range(4):
            for n in range(2):
                ps = ppool.tile([128, LQ], F32, tag="pp", name="ps")
                for u in range(2):
                    for t in range(NKT):
                        nc.tensor.matmul(
                            ps[:, 512 * u:512 * (u + 1)],
                            wk_sb[t][:, 128 * m:128 * (m + 1)],
                            x_sb[t][:, LQ * n + 512 * u:LQ * n + 512 * (u + 1)],
                            start=(t == 0), stop=(t == NKT - 1),
                        )
                nc.scalar.activation(
                    k_sb[m][:, LQ * n:LQ * (n + 1)], ps,
                    mybir.ActivationFunctionType.Identity,
                    bias=bk_sb[:, m:m + 1],
                )

        # V projection, natural layout: out[r', ch]; lhsT = xT r'-slice
        wv_sb = []
        for t in range(NKT):
            wt = pact.tile([128, CH], F16, tag=f"w{t}", name="wt")
            nc.sync.dma_start(wt, wvT[128 * t:128 * (t + 1), :])
            wv_sb.append(wt)
        for j in range(16):         # r' tiles of 128
            ps3 = ppv.tile([128, HPC, DH], F32, tag="ppv", name="ps3")
            for t in range(NKT):
                nc.tensor.matmul(
                    ps3,
                    x_sb[t][:, 128 * j:128 * (j + 1)],
                    wv_sb[t],
                    start=(t == 0), stop=(t == NKT - 1),
                )
            nc.vector.tensor_add(v_sb[:, j, :, 0:DH], ps3, bv_sb)

        # ---------------- phase boundary ----------------
        proj.close()
        with tc.tile_critical():
            nc.all_engine_barrier()

        # ---------------- Phase 2: attention per head ----------------
        qe_dram = ctx.enter_context(tc.tile_pool(name="qed", bufs=16, space="DRAM"))
        ke_dram = ctx.enter_context(tc.tile_pool(name="ked", bufs=32, space="DRAM"))
        relp = ctx.enter_context(tc.tile_pool(name="relp", bufs=3))
        rel1p = ctx.enter_context(tc.tile_pool(name="rel1p", bufs=10))
        r2p = ctx.enter_context(tc.tile_pool(name="r2p", bufs=40))
        ptp = ctx.enter_context(tc.tile_pool(name="ptp", bufs=3))
        cnp = ctx.enter_context(tc.tile_pool(name="cnp", bufs=2))
        qeps = ctx.enter_context(tc.tile_pool(name="qeps", bufs=3, space="PSUM"))
        sps = ctx.enter_context(tc.tile_pool(name="sps", bufs=3, space="PSUM"))
        cps = ctx.enter_context(tc.tile_pool(name="cps", bufs=1, space="PSUM"))

        def build_head(h):
            """Emit QE/KE table builds + rel1/rel2 read-backs for head h.
            Called one head ahead of the score loop; each block's read-back
            is issued right after its write so the round-trip streams."""
            m, hb = h // 2, h % 2
            b0 = 64 * hb            # partition base of this head's q/k rows
            qh = q_sb[m]
            kh = k_sb[m]

            # QE blocks: QE[l, t] fp16, l-block i holds t-window [l0, l0+2175);
            # rel1 cast-read (fp16 -> fp32, skew stride QW+1) follows each write
            rel1_sb = []
            for i in range(8):
                l0 = 128 * i
                qe_sb = relp.tile([128, QW], F16, tag="qe_sb", name="qe_sb")
                for c, w in ((0, 512), (512, 512), (1024, 512), (1536, 512), (2048, 127)):
                    ps = qeps.tile([128, 512], F32, tag="qeps", name="ps")
                    nc.tensor.matmul(
                        ps[:, 0:w],
                        qh[b0:b0 + 64, l0:l0 + 128],
                        et_sb[b0:b0 + 64, l0 + c:l0 + c + w],
                        start=True, stop=True,
                    )
                    if c < 1536:
                        nc.vector.tensor_copy(qe_sb[:, c:c + w], ps[:, 0:w])
                    else:
                        nc.scalar.activation(
                            qe_sb[:, c:c + w], ps[:, 0:w],
                            mybir.ActivationFunctionType.Identity,
                        )
                qe_t = qe_dram.tile([128, QW], F16, tag="qe_d", name="qe_t")
                nc.sync.dma_start(qe_t[:, 0:QW - 1], qe_sb[:, 0:QW - 1])
                t1 = rel1p.tile([128, LK], F32, tag="rel1", name="t1")
                src = bass.AP(
                    tensor=qe_t.tensor,
                    offset=qe_t.offset,
                    ap=[[QW + 1, 128], [1, LK]],
                )
                nc.gpsimd.dma_start(out=t1, in_=src)
                rel1_sb.append(t1)

            # KE blocks: KE[r', t] fp8, window [r0, r0+1151); both rel2^T
            # halves (skew stride KW+1) follow each write
            r2_tiles = []
            for j in range(16):
                r0 = 128 * j
                ke_sb = relp.tile([128, KW], F8, tag="ke_sb", name="ke_sb")
                for c, w in ((0, 512), (512, 512), (1024, 127)):
                    ps = qeps.tile([128, 512], F32, tag="qeps", name="ps")
                    nc.tensor.matmul(
                        ps[:, 0:w],
                        kh[b0:b0 + 64, r0:r0 + 128],
                        es_sb[b0:b0 + 64, r0 + c:r0 + c + w],
                        start=True, stop=True,
                    )
                    if c < 1024:
                        nc.vector.tensor_copy(ke_sb[:, c:c + w], ps[:, 0:w])
                    else:
                        nc.scalar.activation(
                            ke_sb[:, c:c + w], ps[:, 0:w],
                            mybir.ActivationFunctionType.Identity,
                        )
                ke_t = ke_dram.tile([128, KW], F8, tag="ke_d", name="ke_t")
                nc.sync.dma_start(ke_t, ke_sb)
                for lh in range(2):
                    r2 = r2p.tile([128, 512], F8, tag="r2", name="r2")
                    src = bass.AP(
                        tensor=ke_t.tensor,
                        offset=ke_t.offset + 512 * lh,
                        ap=[[KW + 1, 128], [1, 512]],
                    )
                    nc.sync.dma_start(out=r2, in_=src)
                    r2_tiles.append(r2)
            return rel1_sb, r2_tiles

        def score_head(h, built):
            m, hb = h // 2, h % 2
            b0 = 64 * hb
            qh = q_sb[m]
            kh = k_sb[m]
            rel1_sb, r2_tiles = built

            # ctx accumulates in two 1-bank halves, each copied to SBUF as
            # soon as its lh-loop finishes so the next head never waits on
            # this head's ctx drain (cps ring of 2).
            cn_sb = cnp.tile([80, LQ], F16, tag="ctxn", name="cn_sb")
            for lh in range(2):
                ctx_ps = cps.tile([DH + 1, 512], F32, tag="ctxps", name="ctx_ps")
                for j in range(16):
                    s_ps = sps.tile([128, 512], F32, tag="sps", name="s_ps")
                    # QK^T: [r' 128, l 512]
                    nc.tensor.matmul(
                        s_ps,
                        kh[b0:b0 + 64, 128 * j:128 * (j + 1)],
                        qh[b0:b0 + 64, 512 * lh:512 * (lh + 1)],
                        start=True, stop=False,
                    )
                    # rel1: PE-transpose-accumulate 4 blocks of this l-half
                    for ii in range(4):
                        i = 4 * lh + ii
                        nc.tensor.matmul(
                            s_ps[:, 128 * ii:128 * (ii + 1)],
                            rel1_sb[i][:, 128 * j:128 * (j + 1)],
                            ident_f,
                            is_transpose=True,
                            start=False, stop=False,
                        )
                    # rel2: added into PSUM via I @ r2 on the PE
                    nc.tensor.matmul(
                        s_ps, ident_8, r2_tiles[2 * j + lh],
                        start=False, stop=True,
                    )
                    # probs = exp(scores/S + mask[r']), mask per-partition
                    pt = ptp.tile([128, 512], F16, tag="pt", name="pt")
                    nc.scalar.activation(
                        pt, s_ps, mybir.ActivationFunctionType.Exp,
                        bias=mask_sb[:, j:j + 1], scale=1.0 / SCALE,
                    )
                    nc.tensor.matmul(
                        ctx_ps,
                        v_sb[:, j, h, :],
                        pt,
                        start=(j == 0), stop=(j == 15),
                    )
                nc.vector.tensor_copy(
                    cn_sb[0:DH + 1, 512 * lh:512 * (lh + 1)], ctx_ps
                )
            return cn_sb

        def ctx_phase(h, cn_sb):
            """ctx+rowsum transpose/normalize for head h. Emitted AFTER the
            next head's table builds so the XBAR-transpose ucode (~1.2us
            each on the sync queue) never delays the build's qe writes —
            nothing downstream depends on these except the final out DMA.
            [65, 128]-chunks transpose to [128, 80] via SBUF->SBUF XBAR
            (80 source partitions to satisfy its 16-row tiling); 1/rowsum
            applied on the ACT copy out."""
            for i in range(8):
                ct = cnp.tile([128, 80], F16, tag="ct", name="ct", bufs=2)
                nc.sync.dma_start_transpose(ct, cn_sb[:, 128 * i:128 * (i + 1)])
                rs_inv = cnp.tile([128, 1], F32, tag="rsinv", name="rs_inv")
                nc.vector.reciprocal(rs_inv, ct[:, DH:DH + 1])
                nc.scalar.activation(
                    ctx_all[:, i, DH * h:DH * (h + 1)], ct[:, 0:DH],
                    mybir.ActivationFunctionType.Copy,
                    scale=rs_inv,
                )

        built = build_head(0)
        for h in range(HPC):
            cn = score_head(h, built)
            built = build_head(h + 1) if h + 1 < HPC else None
            ctx_phase(h, cn)

        nc.sync.dma_start(out.rearrange("(i p) c -> p i c", p=128), ctx_all)

    nc.compile()
    return nc


def make_in_maps(inputs):
    hs = np.asarray(inputs["hidden_states"], np.float32)
    qhs = np.asarray(inputs["query_hidden_states"], np.float32)
    am = np.asarray(inputs["attention_mask"], np.float32)
    Wq = np.asarray(inputs["Wq"], np.float32)
    bq = np.asarray(inputs["bq"], np.float32)
    Wk = np.asarray(inputs["Wk"], np.float32)
    bk = np.asarray(inputs["bk"], np.float32)
    Wv = np.asarray(inputs["Wv"], np.float32)
    bv = np.asarray(inputs["bv"], np.float32)
    de = np.asarray(inputs["dist_emb"], np.float32)

    # E^T tables, duplicated into both partition halves (base-0 and base-64
    # head slots); esT pre-scaled by S/8 for the KE side.
    eT = np.zeros((128, TW), np.float32)
    eT[0:DH, :3071] = de[:3071].T
    eT[DH:128, :3071] = de[:3071].T
    esT = eT * (SCALE / 8.0)

    qs = SCALE / 8.0
    in_maps = []
    for core in range(8):
        b = core // 2
        hg = core % 2
        sl = slice(CH * hg, CH * (hg + 1))
        m16 = {
            "xqT": np.ascontiguousarray(qhs[b].T),
            "xT": np.ascontiguousarray(hs[b].T[:, ::-1]),
            "wqT": np.ascontiguousarray(Wq[sl].T) * qs,
            "wkT": np.ascontiguousarray(Wk[sl].T),
            "wvT": np.ascontiguousarray(Wv[sl].T),
            "eT": eT,
            "esT": esT,
        }
        m32 = {
            "bqv": np.ascontiguousarray(bq[sl]) * qs,
            "bkv": np.ascontiguousarray(bk[sl]),
            "bvv": np.ascontiguousarray(bv[sl]),
            "maskc": np.ascontiguousarray(am[b, 0, 0, ::-1].reshape(16, 128).T),
        }
        m = {k: np.ascontiguousarray(v.astype(np.float16)) for k, v in m16.items()}
        m.update({k: np.ascontiguousarray(v.astype(np.float32)) for k, v in m32.items()})
        in_maps.append(m)
    return in_maps


_CACHED = {}


def kernel(**inputs):
    from concourse.bass_utils import run_bass_kernel_spmd

    if "nc" not in _CACHED:
        _CACHED["nc"] = build_nc()
    nc = _CACHED["nc"]
    in_maps = make_in_maps(inputs)
    res = run_bass_kernel_spmd(nc, in_maps, list(range(8)))
    _CACHED["last_result"] = res
    out = np.zeros((B, LQ, D), np.float32)
    for core in range(8):
        b = core // 2
        hg = core % 2
        out[b, :, CH * hg:CH * (hg + 1)] = res.results[core]["out"]
    return out


# revision 42
# speedup vs baseline: 1.1928x; 1.0482x over previous
"""BertCrossAttention (relative_key_query) Trainium2 kernel.

Full inputs -> full output. Sharding: 8 cores, core c handles batch b=c//2 and
heads [8*(c%2), 8*(c%2)+8). All sharding/slicing/transposition happens on the
host; each core runs an identical Bass program on its own slices.

Math (per core, per head h), with a global score scale S=16 folded in so the
small relative-position tables survive fp8 storage:
  q = (xq @ Wq^T + bq) * S/8        [Lq=1024, 64] fp16   (1/sqrt(64)=1/8)
  k = x @ Wk^T + bk                  [Lk=2048, 64] fp16
  v = x @ Wv^T + bv                  [Lk=2048, 65] fp16 (ones column appended)
  S[l,r'] = q.k + q.E[t] + k.(S/8 E)[t],  t = l + r'   (key axis host-reversed)
  probs = exp(S/S + mask[r'])  (mask via ACT exp bias; softmax denom from the
                                ones column of v)
  out[l, dh] = (probs @ v)[l, dh] / rowsum[l]

All matmul inputs fp16 (1 PE cycle/row vs 4 for fp32); KE stored fp8e4.

Key layout trick: the key axis is REVERSED on the host (r' = 2047 - r), making
t = l + r'. Then with QE[l,t] (fp16) and KE[r',t] (fp8, k.E pre-scaled) stored
in DRAM as dense windowed blocks, both rel-score reads become plain 2D strided
DMAs (row stride = width+1 "skew trick" on flat DRAM):
  rel1[l, r'] = QE[l, l+r']   (cast-read as [l-part, r'-free] fp32 tiles, then
                               PE-transpose-accumulated onto QK scores in PSUM)
  rel2^T[r', l] = KE[r', l+r'] (read directly as [r'-part, l-free] fp8 tiles,
                               added into PSUM via an identity matmul)
Scores are built transposed (S^T [r'-part, l-free]) so PV needs no transpose
of the probabilities. Table builds run one head ahead of the score loop, with
each block's read-back issued immediately after its write so the DRAM
round-trip overlaps the previous head's score matmuls. The final ctx
transposes go through the DMA XBAR (SBUF->SBUF), not PSUM.

q/k/v stay SBUF-resident between projections and attention: projection output
tile m ([128ch, l]) holds heads 2m (partitions 0-63) and 2m+1 (64-127);
per-head matmuls slice at partition base 0/64 (PE quadrant addressing), with
the E tables duplicated into both halves.
"""

import os
import sys
from contextlib import ExitStack

import numpy as np

sys.path.insert(0, "/opt/trn_rl_repo")

import concourse.bass as bass
import concourse.mybir as mybir
import concourse.tile as tile
from concourse import bacc
from concourse.masks import make_identity

F32 = mybir.dt.float32
F16 = mybir.dt.float16
F8 = mybir.dt.float8e4

B, H, DH, D = 4, 16, 64, 1024
LQ, LK = 1024, 2048
HPC = 8              # heads per core
CH = HPC * DH        # 512 output channels per core
TW = 3072            # E-table columns used (t in [0, 3071))
QW = 2176            # QE block storage width (cols 0..2174 used, 2175 pad)
KW = 1151            # KE block storage width (cols 0..1150 used)
NKT = D // 128       # 8 contraction tiles for projections
SCALE = 16.0         # global score scale (pre-folded into q and the KE table)


def build_nc():
    nc = bacc.Bacc("TRN2", target_bir_lowering=False, debug=False, num_devices=8)

    xqT = nc.dram_tensor("xqT", [D, LQ], F16, kind="ExternalInput")
    xT = nc.dram_tensor("xT", [D, LK], F16, kind="ExternalInput")
    wqT = nc.dram_tensor("wqT", [D, CH], F16, kind="ExternalInput")
    wkT = nc.dram_tensor("wkT", [D, CH], F16, kind="ExternalInput")
    wvT = nc.dram_tensor("wvT", [D, CH], F16, kind="ExternalInput")
    bqv = nc.dram_tensor("bqv", [CH], F32, kind="ExternalInput")
    bkv = nc.dram_tensor("bkv", [CH], F32, kind="ExternalInput")
    bvv = nc.dram_tensor("bvv", [CH], F32, kind="ExternalInput")
    eT = nc.dram_tensor("eT", [128, TW], F16, kind="ExternalInput")
    esT = nc.dram_tensor("esT", [128, TW], F16, kind="ExternalInput")
    maskc = nc.dram_tensor("maskc", [128, 16], F32, kind="ExternalInput")
    out = nc.dram_tensor("out", [LQ, CH], F32, kind="ExternalOutput")

    with tile.TileContext(nc) as tc, ExitStack() as ctx:
        const = ctx.enter_context(tc.tile_pool(name="const", bufs=1))
        ident_f = const.tile([128, 128], F32)
        make_identity(nc, ident_f)
        ident_8 = const.tile([128, 128], F8)
        make_identity(nc, ident_8)
        # E tables, duplicated into both partition halves (host-prepared)
        et_sb = const.tile([128, TW], F16, tag="et")
        es_sb = const.tile([128, TW], F16, tag="est")
        nc.sync.dma_start(et_sb, eT[:, :])
        nc.sync.dma_start(es_sb, esT[:, :])
        mask_sb = const.tile([128, 16], F32, tag="mask")
        nc.sync.dma_start(mask_sb, maskc[:, :])
        bq_sb = const.tile([128, 4], F32, tag="bq")
        bk_sb = const.tile([128, 4], F32, tag="bk")
        nc.sync.dma_start(bq_sb, bqv.rearrange("(t p) -> p t", p=128))
        nc.sync.dma_start(bk_sb, bkv.rearrange("(t p) -> p t", p=128))
        bv_sb = const.tile([128, HPC, DH], F32, tag="bv")
        nc.sync.dma_start(
            bv_sb, bass.AP(tensor=bvv, offset=0, ap=[[0, 128], [DH, HPC], [1, DH]])
        )

        # persistent per-core activation tensors (SBUF-resident q/k/v)
        persist = ctx.enter_context(tc.tile_pool(name="persist", bufs=1))
        q_sb = [
            persist.tile([128, LQ], F16, tag=f"q{m}", name=f"q_sb{m}")
            for m in range(4)
        ]
        k_sb = [
            persist.tile([128, LK], F16, tag=f"k{m}", name=f"k_sb{m}")
            for m in range(4)
        ]
        v_sb = persist.tile([128, 16, HPC, DH + 1], F16, tag="v")  # [r',j,h,dh|1]
        ctx_all = persist.tile([128, 8, CH], F32, tag="ctxo")  # [l%128, lblk, ch]
        nc.vector.memset(v_sb[:, :, :, DH], 1.0)

        # ---------------- Phase 1: projections (fp16) ----------------
        proj = ExitStack()
        ppool = proj.enter_context(tc.tile_pool(name="pp", bufs=2, space="PSUM"))
        ppv = proj.enter_context(tc.tile_pool(name="ppv", bufs=2, space="PSUM"))
        pact = proj.enter_context(tc.tile_pool(name="pact", bufs=1))

        xq_sb, wq_sb = [], []
        for t in range(NKT):
            xt_full = pact.tile([128, LK], F16, tag=f"x{t}", name="xt_full")
            xt = xt_full[:, 0:LQ]
            wt = pact.tile([128, CH], F16, tag=f"w{t}", name="wt")
            nc.sync.dma_start(xt, xqT[128 * t:128 * (t + 1), :])
            nc.sync.dma_start(wt, wqT[128 * t:128 * (t + 1), :])
            xq_sb.append(xt)
            wq_sb.append(wt)

        # Q projection (matmul outputs are capped at 512 fp32 elements, so
        # each 1024-wide PSUM tile fills in halves; one bias-copy drains it)
        for m in range(4):
            ps = ppool.tile([128, LQ], F32, tag="pp", name="ps")
            for n in range(2):
                for t in range(NKT):
                    nc.tensor.matmul(
                        ps[:, 512 * n:512 * (n + 1)],
                        wq_sb[t][:, 128 * m:128 * (m + 1)],
                        xq_sb[t][:, 512 * n:512 * (n + 1)],
                        start=(t == 0), stop=(t == NKT - 1),
                    )
            nc.scalar.activation(
                q_sb[m], ps,
                mybir.ActivationFunctionType.Identity,
                bias=bq_sb[:, m:m + 1],
            )

        x_sb, wk_sb = [], []
        for t in range(NKT):
            xt = pact.tile([128, LK], F16, tag=f"x{t}", name="xt")
            nc.sync.dma_start(xt, xT[128 * t:128 * (t + 1), :])
            x_sb.append(xt)
        for t in range(NKT):
            wt = pact.tile([128, CH], F16, tag=f"w{t}", name="wt")
            nc.sync.dma_start(wt, wkT[128 * t:128 * (t + 1), :])
            wk_sb.append(wt)

        # K projection
        for m in range(4):
            for n in range(2):
                ps = ppool.tile([128, LQ], F32, tag="pp", name="ps")
                for u in range(2):
                    for t in range(NKT):
                        nc.tensor.matmul(
                            ps[:, 512 * u:512 * (u + 1)],
                            wk_sb[t][:, 128 * m:128 * (m + 1)],
                            x_sb[t][:, LQ * n + 512 * u:LQ * n + 512 * (u + 1)],
                            start=(t == 0), stop=(t == NKT - 1),
                        )
                nc.scalar.activation(
                    k_sb[m][:, LQ * n:LQ * (n + 1)], ps,
                    mybir.ActivationFunctionType.Identity,
                    bias=bk_sb[:, m:m + 1],
                )

        # V projection, natural layout: out[r', ch]; lhsT = xT r'-slice
        wv_sb = []
        for t in range(NKT):
            wt = pact.tile([128, CH], F16, tag=f"w{t}", name="wt")
            nc.sync.dma_start(wt, wvT[128 * t:128 * (t + 1), :])
            wv_sb.append(wt)
        for j in range(16):         # r' tiles of 128
            ps3 = ppv.tile([128, HPC, DH], F32, tag="ppv", name="ps3")
            for t in range(NKT):
                nc.tensor.matmul(
                    ps3,
                    x_sb[t][:, 128 * j:128 * (j + 1)],
                    wv_sb[t],
                    start=(t == 0), stop=(t == NKT - 1),
                )
            nc.vector.tensor_add(v_sb[:, j, :, 0:DH], ps3, bv_sb)

        # ---------------- phase boundary ----------------
        proj.close()
        with tc.tile_critical():
            nc.all_engine_barrier()

        # ---------------- Phase 2: attention per head ----------------
        qe_dram = ctx.enter_context(tc.tile_pool(name="qed", bufs=16, space="DRAM"))
        ke_dram = ctx.enter_context(tc.tile_pool(name="ked", bufs=32, space="DRAM"))
        relp = ctx.enter_context(tc.tile_pool(name="relp", bufs=3))
        rel1p = ctx.enter_context(tc.tile_pool(name="rel1p", bufs=10))
        r2p = ctx.enter_context(tc.tile_pool(name="r2p", bufs=40))
        ptp = ctx.enter_context(tc.tile_pool(name="ptp", bufs=3))
        cnp = ctx.enter_context(tc.tile_pool(name="cnp", bufs=2))
        qeps = ctx.enter_context(tc.tile_pool(name="qeps", bufs=3, space="PSUM"))
        sps = ctx.enter_context(tc.tile_pool(name="sps", bufs=3, space="PSUM"))
        cps = ctx.enter_context(tc.tile_pool(name="cps", bufs=1, space="PSUM"))

        def build_head(h):
            """Emit QE/KE table builds + rel1/rel2 read-backs for head h.
            Called one head ahead of the score loop; each block's read-back
            is issued right after its write so the round-trip streams."""
            m, hb = h // 2, h % 2
            b0 = 64 * hb            # partition base of this head's q/k rows
            qh = q_sb[m]
            kh = k_sb[m]

            # QE blocks: QE[l, t] fp16, l-block i holds t-window [l0, l0+2175);
            # rel1 cast-read (fp16 -> fp32, skew stride QW+1) follows each write
            rel1_sb = []
            for i in range(8):
                l0 = 128 * i
                qe_sb = relp.tile([128, QW], F16, tag="qe_sb", name="qe_sb")
                for c, w in ((0, 512), (512, 512), (1024, 512), (1536, 512), (2048, 127)):
                    ps = qeps.tile([128, 512], F32, tag="qeps", name="ps")
                    nc.tensor.matmul(
                        ps[:, 0:w],
                        qh[b0:b0 + 64, l0:l0 + 128],
                        et_sb[b0:b0 + 64, l0 + c:l0 + c + w],
                        start=True, stop=True,
                    )
                    if c < 1536:
                        nc.vector.tensor_copy(qe_sb[:, c:c + w], ps[:, 0:w])
                    else:
                        nc.scalar.activation(
                            qe_sb[:, c:c + w], ps[:, 0:w],
                            mybir.ActivationFunctionType.Identity,
                        )
                qe_t = qe_dram.tile([128, QW], F16, tag="qe_d", name="qe_t")
                nc.sync.dma_start(qe_t[:, 0:QW - 1], qe_sb[:, 0:QW - 1])
                t1 = rel1p.tile([128, LK], F32, tag="rel1", name="t1")
                src = bass.AP(
                    tensor=qe_t.tensor,
                    offset=qe_t.offset,
                    ap=[[QW + 1, 128], [1, LK]],
                )
                nc.gpsimd.dma_start(out=t1, in_=src)
                rel1_sb.append(t1)

            # KE blocks: KE[r', t] fp8, window [r0, r0+1151); both rel2^T
            # halves (skew stride KW+1) follow each write
            r2_tiles = []
            for j in range(16):
                r0 = 128 * j
                ke_sb = relp.tile([128, KW], F8, tag="ke_sb", name="ke_sb")
                for c, w in ((0, 512), (512, 512), (1024, 127)):
                    ps = qeps.tile([128, 512], F32, tag="qeps", name="ps")
                    nc.tensor.matmul(
                        ps[:, 0:w],
                        kh[b0:b0 + 64, r0:r0 + 128],
                        es_sb[b0:b0 + 64, r0 + c:r0 + c + w],
                        start=True, stop=True,
                    )
                    if c < 1024:
                        nc.vector.tensor_copy(ke_sb[:, c:c + w], ps[:, 0:w])
                    else:
                        nc.scalar.activation(
                            ke_sb[:, c:c + w], ps[:, 0:w],
                            mybir.ActivationFunctionType.Identity,
                        )
                ke_t = ke_dram.tile([128, KW], F8, tag="ke_d", name="ke_t")
                nc.sync.dma_start(ke_t, ke_sb)
                for lh in range(2):
                    r2 = r2p.tile([128, 512], F8, tag="r2", name="r2")
                    src = bass.AP(
                        tensor=ke_t.tensor,
                        offset=ke_t.offset + 512 * lh,
                        ap=[[KW + 1, 128], [1, 512]],
                    )
                    nc.sync.dma_start(out=r2, in_=src)
                    r2_tiles.append(r2)
            return rel1_sb, r2_tiles

        def score_head(h, built):
            m, hb = h // 2, h % 2
            b0 = 64 * hb
            qh = q_sb[m]
            kh = k_sb[m]
            rel1_sb, r2_tiles = built

            ctx_ps = cps.tile([DH + 1, LQ], F32, tag="ctxps", name="ctx_ps")
            for lh in range(2):
                for j in range(16):
                    s_ps = sps.tile([128, 512], F32, tag="sps", name="s_ps")
                    # QK^T: [r' 128, l 512]
                    nc.tensor.matmul(
                        s_ps,
                        kh[b0:b0 + 64, 128 * j:128 * (j + 1)],
                        qh[b0:b0 + 64, 512 * lh:512 * (lh + 1)],
                        start=True, stop=False,
                    )
                    # rel1: PE-transpose-accumulate 4 blocks of this l-half
                    for ii in range(4):
                        i = 4 * lh + ii
                        nc.tensor.matmul(
                            s_ps[:, 128 * ii:128 * (ii + 1)],
                            rel1_sb[i][:, 128 * j:128 * (j + 1)],
                            ident_f,
                            is_transpose=True,
                            start=False, stop=False,
                        )
                    # rel2: added into PSUM via I @ r2 on the PE
                    nc.tensor.matmul(
                        s_ps, ident_8, r2_tiles[2 * j + lh],
                        start=False, stop=True,
                    )
                    # probs = exp(scores/S + mask[r']), mask per-partition
                    pt = ptp.tile([128, 512], F16, tag="pt", name="pt")
                    nc.scalar.activation(
                        pt, s_ps, mybir.ActivationFunctionType.Exp,
                        bias=mask_sb[:, j:j + 1], scale=1.0 / SCALE,
                    )
                    nc.tensor.matmul(
                        ctx_ps[:, 512 * lh:512 * (lh + 1)],
                        v_sb[:, j, h, :],
                        pt,
                        start=(j == 0), stop=(j == 15),
                    )

            # ctx+rowsum -> SBUF fp16 (80 partitions so the XBAR transpose's
            # 16-row tiling works); transpose [65, 128]-chunks to [128, 80]
            # via SBUF->SBUF XBAR; 1/rowsum applied on the ACT copy out.
            cn_sb = cnp.tile([80, LQ], F16, tag="ctxn", name="cn_sb")
            nc.vector.tensor_copy(cn_sb[0:DH + 1, :], ctx_ps)
            for i in range(8):
                ct = cnp.tile([128, 80], F16, tag="ct", name="ct", bufs=2)
                nc.sync.dma_start_transpose(ct, cn_sb[:, 128 * i:128 * (i + 1)])
                rs_inv = cnp.tile([128, 1], F32, tag="rsinv", name="rs_inv")
                nc.vector.reciprocal(rs_inv, ct[:, DH:DH + 1])
                nc.scalar.activation(
                    ctx_all[:, i, DH * h:DH * (h + 1)], ct[:, 0:DH],
                    mybir.ActivationFunctionType.Copy,
                    scale=rs_inv,
                )

        built = build_head(0)
        for h in range(HPC):
            nxt = build_head(h + 1) if h + 1 < HPC else None
            score_head(h, built)
            built = nxt

        nc.sync.dma_start(out.rearrange("(i p) c -> p i c", p=128), ctx_all)

    nc.compile()
    return nc


def make_in_maps(inputs):
    hs = np.asarray(inputs["hidden_states"], np.float32)
    qhs = np.asarray(inputs["query_hidden_states"], np.float32)
    am = np.asarray(inputs["attention_mask"], np.float32)
    Wq = np.asarray(inputs["Wq"], np.float32)
    bq = np.asarray(inputs["bq"], np.float32)
    Wk = np.asarray(inputs["Wk"], np.float32)
    bk = np.asarray(inputs["bk"], np.float32)
    Wv = np.asarray(inputs["Wv"], np.float32)
    bv = np.asarray(inputs["bv"], np.float32)
    de = np.asarray(inputs["dist_emb"], np.float32)

    # E^T tables, duplicated into both partition halves (base-0 and base-64
    # head slots); esT pre-scaled by S/8 for the KE side.
    eT = np.zeros((128, TW), np.float32)
    eT[0:DH, :3071] = de[:3071].T
    eT[DH:128, :3071] = de[:3071].T
    esT = eT * (SCALE / 8.0)

    qs = SCALE / 8.0
    in_maps = []
    for core in range(8):
        b = core // 2
        hg = core % 2
        sl = slice(CH * hg, CH * (hg + 1))
        m16 = {
            "xqT": np.ascontiguousarray(qhs[b].T),
            "xT": np.ascontiguousarray(hs[b].T[:, ::-1]),
            "wqT": np.ascontiguousarray(Wq[sl].T) * qs,
            "wkT": np.ascontiguousarray(Wk[sl].T),
            "wvT": np.ascontiguousarray(Wv[sl].T),
            "eT": eT,
            "esT": esT,
        }
        m32 = {
            "bqv": np.ascontiguousarray(bq[sl]) * qs,
            "bkv": np.ascontiguousarray(bk[sl]),
            "bvv": np.ascontiguousarray(bv[sl]),
            "maskc": np.ascontiguousarray(am[b, 0, 0, ::-1].reshape(16, 128).T),
        }
        m = {k: np.ascontiguousarray(v.astype(np.float16)) for k, v in m16.items()}
        m.update({k: np.ascontiguousarray(v.astype(np.float32)) for k, v in m32.items()})
        in_maps.append(m)
    return in_maps


_CACHED = {}


def kernel(**inputs):
    from concourse.bass_utils import run_bass_kernel_spmd

    if "nc" not in _CACHED:
        _CACHED["nc"] = build_nc()
    nc = _CACHED["nc"]
    in_maps = make_in_maps(inputs)
    res = run_bass_kernel_spmd(nc, in_maps, list(range(8)))
    _CACHED["last_result"] = res
    out = np.zeros((B, LQ, D), np.float32)
    for core in range(8):
        b = core // 2
        hg = core % 2
        out[b, :, CH * hg:CH * (hg + 1)] = res.results[core]["out"]
    return out
